# revision 1
# baseline (speedup 1.0000x reference)
"""BiAttention (BiDAF) Trainium2 Bass kernel — 8 NeuronCores, sequence-
parallel over the context axis.

kernel(context [16384,100] f32, question [4096,100] f32, kernel [300] f32)
  -> G [16384, 400] f32  (concat: ctx | U_A | ctx*U_A | ctx*H_A)

Per core (2048 context rows): S is computed twice in fp32r (1 cyc/row on the
PE): once ctx-major for the exact row-max (DVE reduce), once q-major with the
row-max folded in as an extra contraction row, so ACT exp reads S-m straight
from PSUM. U_A^T accumulates on the PE with a ones-column giving the softmax
denominator for free. The Q2C softmax over row-maxes uses one 102-float
AllGather, overlapped with the second pass.
"""
import sys

sys.path.insert(0, "/opt/trn_rl_repo")
from contextlib import ExitStack

import numpy as np

import concourse.bass as bass
import concourse.tile as tile
from concourse import mybir


def split_multi_waits(nc):
    """This walrus build rejects instructions with >1 sync wait. Hoist extra
    waits onto single-wait EventSemaphore nops on the same engine (engines
    execute in order, so N sequential single waits == one N-way wait)."""
    n_split = 0
    counter = [0]

    def make_nop(engine, wait):
        counter[0] += 1
        inst = mybir.InstEventSemaphore(
            name=f"I-waitsplit-{counter[0]}", ins=[], outs=[])
        inst.engine = engine
        inst.sync_info = mybir.SyncInfo(on_wait=[wait], on_update=[])
        return inst

    for f in nc.m.functions:
        for blk in f.blocks:
            changed = False
            new_insts = []
            for inst in blk.instructions:
                si = inst.sync_info
                if si is not None and si.on_wait and len(si.on_wait) > 1:
                    waits = list(si.on_wait)
                    for w in waits[:-1]:
                        new_insts.append(make_nop(inst.engine, w))
                    si.on_wait = [waits[-1]]
                    n_split += 1
                    changed = True
                new_insts.append(inst)
            if changed:
                blk.instructions[:] = new_insts
    return n_split


F32 = mybir.dt.float32
F32R = mybir.dt.float32r
EXP = mybir.ActivationFunctionType.Exp
COPY = mybir.ActivationFunctionType.Copy

N_CORES = 8
D = 100
R = 2048          # ctx rows per core
M = 4096          # question rows
P = 128           # partitions
NCH = R // P      # 16 ctx chunks
QC = M // P       # 32 q chunks
JT = M // 512     # 8 q tiles (pass B)
NT = R // 512     # 4 ctx tiles (pass C)
CPT = 512 // P    # 4 chunks per ctx tile


def build_bass():
    nc = bass.Bass("TRN2", target_bir_lowering=False, debug=False,
                   num_devices=N_CORES)
    ctx_in = nc.dram_tensor("ctx", [R, D], F32, kind="ExternalInput").ap()
    ctxTa_in = nc.dram_tensor("ctxTa", [104, R], F32, kind="ExternalInput").ap()
    qaugTa_in = nc.dram_tensor("qaugTa", [104, M], F32, kind="ExternalInput").ap()
    qnr_in = nc.dram_tensor("qnr", [M, D], F32, kind="ExternalInput").ap()
    id_in = nc.dram_tensor("ident", [P, P], F32, kind="ExternalInput").ap()
    g_out = nc.dram_tensor("g", [R, 4 * D], F32, kind="ExternalOutput").ap()

    with tile.TileContext(nc) as tc:
        with ExitStack() as ex:
            build_body(nc, tc, ex, ctx_in, ctxTa_in, qaugTa_in, qnr_in,
                       id_in, g_out)
    return nc


def build_body(nc, tc, ex, ctx_in, ctxTa_in, qaugTa_in, qnr_in, id_in, g_out):
    sing = ex.enter_context(tc.tile_pool(name="sing", bufs=1))
    pt_pool = ex.enter_context(tc.tile_pool(name="pt", bufs=4))
    uat_pool = ex.enter_context(tc.tile_pool(name="uat", bufs=2))
    g12_pool = ex.enter_context(tc.tile_pool(name="g12", bufs=3))
    row_pool = ex.enter_context(tc.tile_pool(name="rowst", bufs=2))
    # PSUM: B 1x[128,2048](4 banks) + ST 2x[128,512](2) + UA 1x[101,512](1)
    # + tiny 1 bank = 8
    bp = ex.enter_context(tc.tile_pool(name="bp", bufs=2, space="PSUM"))
    stp = ex.enter_context(tc.tile_pool(name="stp", bufs=2, space="PSUM"))
    uap = ex.enter_context(tc.tile_pool(name="uap", bufs=1, space="PSUM"))
    tp = ex.enter_context(tc.tile_pool(name="tp", bufs=1, space="PSUM"))
    dram = ex.enter_context(tc.tile_pool(name="dram", bufs=1, space="DRAM"))

    # ---- persistent SBUF ----
    caugT = sing.tile([104, R], F32R)     # 0..99 ctxT | 100 ones | 101 c1 | 102 -m
    qaugT = sing.tile([104, M], F32R)     # 0..99 qT*w3 | 100 q2 | 101 ones | 102 ones
    qaugN = sing.tile([P, QC, 104], F32R)  # q natural chunks + ones col
    ctxn = sing.tile([P, NCH, 104], F32)   # ctx natural chunks (fp32, for G muls)
    ctxnr = sing.tile([P, NCH, 104], F32)  # ctx natural + ones col (hl lhsT)
    tid = sing.tile([P, P], F32)
    mstore = sing.tile([P, NCH], F32)
    uan = sing.tile([P, NCH, 104], F32)    # UA unnorm natural + Z col
    rzs = sing.tile([P, NCH], F32)         # 1/Z per chunk
    ones1 = sing.tile([1, P], F32)
    hB = sing.tile([P, D], F32)
    g3big = sing.tile([P, NCH, D], F32)
    dummy = sing.tile([1, 1], F32)

    cc_in = dram.tile([1, 102], F32)
    cc_out = dram.tile([N_CORES, 102], F32)

    # ---- input loads (critical first: caugT, qaugT feed pass B) ----
    stg_c = sing.tile([104, R], F32)
    stg_q = sing.tile([104, M], F32)
    stg_n = sing.tile([P, QC, 104], F32)
    nc.sync.dma_start(out=tid[:], in_=id_in[:])
    nc.sync.dma_start(out=stg_q[:], in_=qaugTa_in[:])
    nc.sync.dma_start(out=stg_c[:], in_=ctxTa_in[:])
    nc.vector.tensor_copy(caugT[0:102, :], stg_c[0:102, :])
    nc.vector.tensor_copy(qaugT[0:103, :], stg_q[0:103, :])
    nc.vector.memset(stg_n[:, :, 100:104], 1.0)
    nc.sync.dma_start(
        out=stg_n[:, :, 0:D],
        in_=qnr_in.rearrange("(c p) d -> p c d", p=P))
    nc.vector.tensor_copy(qaugN[:], stg_n[:])
    nc.vector.memset(ctxnr[:, :, 100:101], 1.0)
    nc.sync.dma_start(
        out=ctxn[:, :, 0:D],
        in_=ctx_in.rearrange("(c p) d -> p c d", p=P))
    nc.sync.dma_start(
        out=ctxnr[:, :, 0:D],
        in_=ctx_in.rearrange("(c p) d -> p c d", p=P))
    nc.vector.memset(ones1[:], 1.0)
    nc.vector.memset(dummy[:], 0.0)
    # preload the exp table set early (hidden behind input DMAs)
    nc.scalar.activation(dummy[:], dummy[:], EXP)

    # G cols 0:100 = context verbatim (DRAM->DRAM)
    nc.sync.dma_start(out=g_out[:, 0:D], in_=ctx_in[:])

    def do_b(t):
        for ci in range(CPT):
            cc = t * CPT + ci
            lhs = caugT[0:102, cc * P:(cc + 1) * P]
            qtmp = [sing.tile([P, 1], F32, tag=f"btmp{q}", name=f"btmp{q}_{cc}")
                    for q in range(4)]
            for quarter in range(4):
                sp = bp.tile([P, 1024], F32)
                for j in range(2):
                    joff = (quarter * 2 + j) * 512
                    nc.tensor.matmul(sp[:, j * 512:(j + 1) * 512], lhs,
                                     qaugT[0:102, joff:joff + 512],
                                     start=True, stop=True)
                nc.vector.reduce_max(qtmp[quarter][:], sp[:],
                                     axis=mybir.AxisListType.X)
            nc.vector.tensor_max(qtmp[0][:], qtmp[0][:], qtmp[1][:])
            nc.vector.tensor_max(qtmp[2][:], qtmp[2][:], qtmp[3][:])
            nc.vector.tensor_max(mstore[:, cc:cc + 1], qtmp[0][:], qtmp[2][:])
        mneg = sing.tile([P, CPT], F32, tag="mneg")
        nc.scalar.mul(mneg[:], mstore[:, t * CPT:(t + 1) * CPT], -1.0)
        rowps = tp.tile([1, 512], F32, tag="tiny")
        for ci in range(CPT):
            nc.tensor.transpose(rowps[:, ci * P:(ci + 1) * P],
                                mneg[:, ci:ci + 1], tid[:])
        mst = row_pool.tile([1, 512], F32R, tag="rowstage")
        nc.scalar.activation(mst[:], rowps[:], COPY)
        nc.gpsimd.dma_start(out=caugT[102:103, t * 512:(t + 1) * 512], in_=mst[:])

    def do_c(t):
        first_mul = [None]
        uaps = uap.tile([101, 512], F32)
        for qc in range(QC):
            stps = stp.tile([P, 512], F32, tag="stps")
            nc.tensor.matmul(stps[:], qaugT[0:103, qc * P:(qc + 1) * P],
                             caugT[0:103, t * 512:(t + 1) * 512],
                             start=True, stop=True)
            ptt = pt_pool.tile([P, 512], F32R, tag="ptt")
            nc.scalar.activation(ptt[:], stps[:], EXP)
            nc.tensor.matmul(uaps[:], qaugN[:, qc, 0:101], ptt[:],
                             start=(qc == 0), stop=(qc == QC - 1))
        uat = uat_pool.tile([101, 512], F32)
        nc.vector.tensor_copy(uat[:], uaps[:])
        for ci in range(CPT):
            cc = t * CPT + ci
            uanps = tp.tile([P, 101], F32, tag="tiny")
            nc.tensor.transpose(uanps[:], uat[:, ci * P:(ci + 1) * P],
                                tid[0:101, 0:101])
            nc.vector.tensor_copy(uan[:, cc, 0:101], uanps[:])
            nc.vector.reciprocal(rzs[:, cc:cc + 1], uan[:, cc, 100:101])
            g12 = g12_pool.tile([P, 2 * D], F32, tag="g12")
            m1 = nc.gpsimd.tensor_scalar_mul(g12[:, 0:D], uan[:, cc, 0:D],
                                             rzs[:, cc:cc + 1])
            if first_mul[0] is None:
                first_mul[0] = m1
            nc.gpsimd.tensor_mul(g12[:, D:2 * D], ctxn[:, cc, 0:D], g12[:, 0:D])
            last = nc.sync.dma_start(out=g_out[cc * P:(cc + 1) * P, D:3 * D],
                                     in_=g12[:])
        return last, first_mul[0]

    def do_partials():
        lm1 = sing.tile([P, 1], F32)
        nc.vector.reduce_max(lm1[:], mstore[:], axis=mybir.AxisListType.X)
        lrps = tp.tile([1, P], F32, tag="tiny")
        nc.tensor.transpose(lrps[:], lm1[:], tid[:])
        lrow = sing.tile([1, P], F32)
        nc.scalar.activation(lrow[:], lrps[:], COPY)
        lmax = sing.tile([1, 1], F32)
        nc.vector.reduce_max(lmax[:], lrow[:], axis=mybir.AxisListType.X)
        nlm = sing.tile([1, 1], F32)
        nc.scalar.mul(nlm[:], lmax[:], -1.0)
        nbps = tp.tile([P, 1], F32, tag="tiny")
        nc.tensor.matmul(nbps[:], ones1[:], nlm[:], start=True, stop=True)
        negb = sing.tile([P, 1], F32)
        nc.scalar.activation(negb[:], nbps[:], COPY)
        ee = sing.tile([P, NCH], F32)
        nc.scalar.activation(ee[:], mstore[:], EXP, bias=negb[:])
        hlps = tp.tile([101, 1], F32, tag="tiny")
        for cc in range(NCH):
            nc.tensor.matmul(hlps[:], ctxnr[:, cc, 0:101], ee[:, cc:cc + 1],
                             start=(cc == 0), stop=(cc == NCH - 1))
        hl = sing.tile([101, 1], F32)
        nc.vector.tensor_copy(hl[:], hlps[:])
        nc.gpsimd.dma_start(
            out=cc_in[0:1, 0:101].rearrange("one k -> k one"), in_=hl[:])
        nc.gpsimd.dma_start(out=cc_in[0:1, 101:102], in_=lmax[:])
        return nc.gpsimd.collective_compute(
            "AllGather", mybir.AluOpType.bypass,
            replica_groups=[list(range(N_CORES))],
            ins=[cc_in.opt()], outs=[cc_out.opt()])

    # software-pipelined order: collective launches after B3 and is
    # hidden behind C2/C3
    from concourse.tile_rust import add_dep_helper as _adh0
    do_b(0)
    do_b(1)
    do_c(0)
    do_b(2)
    do_c(1)
    do_b(3)
    cc_inst = do_partials()
    _, c2_mul = do_c(2)
    c3_last, _ = do_c(3)
    # ordering-only edge: keep the AllGather trigger ahead of C2's gpsimd work
    _adh0(c2_mul.ins, cc_inst.ins, sync=False, reason="collective before C2 muls")

    # ---- combine after AllGather ----
    # Pin the combine's first loads behind C3's last store so the scheduler
    # cannot slot the collective-waiting ops into idle engines mid-C (which
    # would stall the C pipeline behind the AllGather).
    from concourse.tile_rust import add_dep_helper as _adh
    agm = sing.tile([N_CORES, 102], F32)
    d1 = nc.sync.dma_start(out=agm[:], in_=cc_out[:])
    lr8 = sing.tile([1, N_CORES], F32)
    d2 = nc.sync.dma_start(out=lr8[:],
                      in_=cc_out[:, 101:102].rearrange("k one -> one k"))
    _adh(d1.ins, c3_last.ins, sync=True, reason="combine after C3")
    _adh(d2.ins, c3_last.ins, sync=True, reason="combine after C3")
    gmax = sing.tile([1, 1], F32)
    nc.vector.reduce_max(gmax[:], lr8[:], axis=mybir.AxisListType.X)
    ngm = sing.tile([1, 1], F32)
    nc.scalar.mul(ngm[:], gmax[:], -1.0)
    srow = sing.tile([1, N_CORES], F32)
    nc.scalar.activation(srow[:], lr8[:], EXP, bias=ngm[:])
    s8ps = tp.tile([N_CORES, 1], F32, tag="tiny")
    nc.tensor.transpose(s8ps[:], srow[:], tid[0:1, 0:1])
    s8 = sing.tile([N_CORES, 1], F32)
    nc.scalar.activation(s8[:], s8ps[:], COPY)
    hsps = tp.tile([1, 102], F32, tag="tiny")
    nc.tensor.matmul(hsps[:], s8[:], agm[:], start=True, stop=True)
    hsum = sing.tile([1, 102], F32)
    nc.scalar.activation(hsum[:], hsps[:], COPY)
    rzh = sing.tile([1, 1], F32)
    nc.vector.reciprocal(rzh[:], hsum[:, 100:101])
    hrow = sing.tile([1, D], F32)
    nc.vector.tensor_scalar_mul(hrow[:], hsum[:, 0:D], rzh[:])
    hbps = tp.tile([P, D], F32, tag="tiny")
    nc.tensor.matmul(hbps[:], ones1[:], hrow[:], start=True, stop=True)
    nc.scalar.activation(hB[:], hbps[:], COPY)
    for cc in range(NCH):
        nc.gpsimd.tensor_mul(g3big[:, cc, :], ctxn[:, cc, 0:D], hB[:])
    nc.sync.dma_start(
        out=g_out[:, 3 * D:4 * D].rearrange("(c p) d -> p c d", p=P),
        in_=g3big[:])


_nc_cache = None


def _get_nc():
    global _nc_cache
    if _nc_cache is None:
        _nc_cache = build_bass()
        split_multi_waits(_nc_cache)
    return _nc_cache


def kernel(**inputs):
    from concourse.bass_utils import run_bass_kernel_spmd

    context = np.ascontiguousarray(inputs["context"], dtype=np.float32)
    question = np.ascontiguousarray(inputs["question"], dtype=np.float32)
    kern = np.ascontiguousarray(inputs["kernel"], dtype=np.float32)
    w1, w2, w3 = kern[:D], kern[D:2 * D], kern[2 * D:]
    q2 = question @ w2
    qaugTa = np.empty((104, question.shape[0]), np.float32)
    qaugTa[0:D] = (question * w3[None, :]).T
    qaugTa[D] = q2
    qaugTa[D + 1:] = 1.0
    qaugTa = np.ascontiguousarray(qaugTa)
    ident = np.eye(128, dtype=np.float32)

    in_maps = []
    for k in range(N_CORES):
        cshard = np.ascontiguousarray(context[k * R:(k + 1) * R])
        ctxTa = np.empty((104, R), np.float32)
        ctxTa[0:D] = cshard.T
        ctxTa[D] = 1.0
        ctxTa[D + 1] = cshard @ w1
        in_maps.append({
            "ctx": cshard,
            "ctxTa": np.ascontiguousarray(ctxTa),
            "qaugTa": qaugTa,
            "qnr": question,
            "ident": ident,
        })
    res = run_bass_kernel_spmd(_get_nc(), in_maps,
                               core_ids=list(range(N_CORES)))
    return np.concatenate([res.results[k]["g"] for k in range(N_CORES)],
                          axis=0)


def kernel_traced(**inputs):
    """Like kernel() but also returns HW exec time in ns (NTFF profile)."""
    from concourse.bass_utils import run_bass_kernel_spmd

    out = kernel(**inputs)  # warm compile via cached nc
    # rerun with trace on the same module
    context = np.ascontiguousarray(inputs["context"], dtype=np.float32)
    question = np.ascontiguousarray(inputs["question"], dtype=np.float32)
    kern = np.ascontiguousarray(inputs["kernel"], dtype=np.float32)
    w1, w2, w3 = kern[:D], kern[D:2 * D], kern[2 * D:]
    q2 = question @ w2
    qaugTa = np.empty((104, question.shape[0]), np.float32)
    qaugTa[0:D] = (question * w3[None, :]).T
    qaugTa[D] = q2
    qaugTa[D + 1:] = 1.0
    ident = np.eye(128, dtype=np.float32)
    in_maps = []
    for k in range(N_CORES):
        cshard = np.ascontiguousarray(context[k * R:(k + 1) * R])
        ctxTa = np.empty((104, R), np.float32)
        ctxTa[0:D] = cshard.T
        ctxTa[D] = 1.0
        ctxTa[D + 1] = cshard @ w1
        in_maps.append({
            "ctx": cshard,
            "ctxTa": np.ascontiguousarray(ctxTa),
            "qaugTa": np.ascontiguousarray(qaugTa),
            "qnr": question,
            "ident": ident,
        })
    res = run_bass_kernel_spmd(_get_nc(), in_maps,
                               core_ids=list(range(N_CORES)), trace=True)
    out = np.concatenate([res.results[k]["g"] for k in range(N_CORES)],
                         axis=0)
    return out, res.exec_time_ns

